# revision 1
# baseline (speedup 1.0000x reference)
"""CompressedLinear Trainium2 kernel.

Computes out[b,s,o] = x[b,s,i] @ (int8_weight[o,i] * scale).T + bias[o]
with x: [4,2048,4096] f32, weight_int8: [11008,4096] int32 (int8 values),
scale: scalar f32, bias: [11008] f32.

Sharding: column-parallel over 8 NeuronCores — each core owns 1376
out-features (weight + bias slice), x is replicated, outputs concat on
the last dim.

Per-core device kernel (Bass/Tile):
  - weight slice is uploaded in [in, out] layout (int32); the device
    dequantizes shard-locally: SWDGE cast-DMA int32 -> bf16 (exact for
    int8-range values) into a resident SBUF tile [4096 x 1376].
  - x is uploaded in [in, s] layout (f32); streamed as SWDGE cast-DMA
    f32 -> bf16 tiles.
  - TensorE: psum[s=128, o<=512] += xT_tile[k,s].T-free @ wT_tile[k,o]
    accumulated over 32 k-tiles of 128.
  - epilogue (DVE): out = psum * scale + bias in one scalar_tensor_tensor,
    then HWDGE store to DRAM in natural [s, o] layout.
"""

import numpy as np

import concourse.bacc as bacc
import concourse.mybir as mybir
import concourse.tile as tile
from concourse.bass_utils import run_bass_kernel_spmd

# Problem shape (hardcoded per contract)
B, S, IN_F, OUT_F = 4, 2048, 4096, 11008
NCORES = 8
OUT_PER = OUT_F // NCORES  # 1376
S_TOT = B * S  # 8192

# Tiling
KTILE = 128  # contraction per matmul
S_CHUNK = 512  # s-columns per x-load group
S_SUB = 128  # out-rows per psum block
KGRP = 4  # k-tiles per x DMA (1 MiB f32 reads)
NMAX = 512  # max moving free dim / psum bank

# set by test harness to capture profiles; harness calls kernel() untouched
TRACE = False
LAST_RESULT = None

_cache = {}


def _n_chunks(out_per):
    chunks = []
    off = 0
    while off < out_per:
        sz = min(NMAX, out_per - off)
        chunks.append((off, sz))
        off += sz
    return chunks


def build_nc(s_tot=S_TOT, in_f=IN_F, out_per=OUT_PER, s_chunk=S_CHUNK, kgrp=KGRP):
    f32 = mybir.dt.float32
    bf16 = mybir.dt.bfloat16
    i32 = mybir.dt.int32

    KT = in_f // KTILE  # k-tiles
    NKG = KT // kgrp  # x-load groups per s-chunk
    chunks = _n_chunks(out_per)

    nc = bacc.Bacc("TRN2", target_bir_lowering=False, debug=False, num_devices=NCORES)

    xt = nc.dram_tensor("xt", [in_f, s_tot], f32, kind="ExternalInput").ap()
    wt = nc.dram_tensor("wt", [in_f, out_per], i32, kind="ExternalInput").ap()
    bias = nc.dram_tensor("bias", [128, out_per], f32, kind="ExternalInput").ap()
    scale = nc.dram_tensor("scale", [128, 1], f32, kind="ExternalInput").ap()
    out = nc.dram_tensor("out", [s_tot, out_per], f32, kind="ExternalOutput").ap()

    with tile.TileContext(nc) as tc:
        with (
            tc.tile_pool(name="wt", bufs=1) as wt_pool,
            tc.tile_pool(name="xbf", bufs=2 * NKG) as xbf_pool,
            tc.tile_pool(name="psum", bufs=2, space="PSUM") as psum_pool,
            tc.tile_pool(name="osb", bufs=3) as osb_pool,
            tc.tile_pool(name="consts", bufs=1) as const_pool,
        ):
            # one-time: dequantize weight slice (int32 -> bf16, exact for int8)
            wts = []
            for k in range(KT):
                t = wt_pool.tile([128, out_per], bf16, tag=f"wt{k}", name=f"wt{k}")
                nc.gpsimd.dma_start(out=t[:], in_=wt[k * 128 : (k + 1) * 128, :])
                wts.append(t)

            bias_sb = const_pool.tile([128, out_per], f32, tag="bias", name="bias_sb")
            nc.sync.dma_start(out=bias_sb[:], in_=bias[:, :])
            scale_sb = const_pool.tile([128, 1], f32, tag="scale", name="scale_sb")
            nc.sync.dma_start(out=scale_sb[:], in_=scale[:, :])

            for ci, s0 in enumerate(range(0, s_tot, s_chunk)):
                # x chunk load: cast f32 -> bf16 in DMA, [128, kgrp, s_chunk]
                xg = []
                for g in range(NKG):
                    t = xbf_pool.tile(
                        [128, kgrp, s_chunk], bf16, tag="xbf", name=f"x{ci}_{g}"
                    )
                    src = xt[
                        g * kgrp * 128 : (g + 1) * kgrp * 128, s0 : s0 + s_chunk
                    ].rearrange("(g p) s -> p g s", p=128)
                    nc.gpsimd.dma_start(out=t[:], in_=src)
                    xg.append(t)

                for sub in range(s_chunk // S_SUB):
                    psums = [
                        psum_pool.tile(
                            [128, NMAX], f32, tag=f"ps{j}", name=f"ps{ci}_{sub}_{j}"
                        )
                        for j in range(len(chunks))
                    ]
                    for k in range(KT):
                        lhsT = xg[k // kgrp][:, k % kgrp, sub * 128 : (sub + 1) * 128]
                        for j, (off, sz) in enumerate(chunks):
                            nc.tensor.matmul(
                                psums[j][:, :sz],
                                lhsT,
                                wts[k][:, off : off + sz],
                                start=(k == 0),
                                stop=(k == KT - 1),
                            )
                    osb = osb_pool.tile(
                        [128, out_per], f32, tag="osb", name=f"o{ci}_{sub}"
                    )
                    for j, (off, sz) in enumerate(chunks):
                        nc.vector.scalar_tensor_tensor(
                            osb[:, off : off + sz],
                            psums[j][:, :sz],
                            scale_sb[:, 0:1],
                            bias_sb[:, off : off + sz],
                            mybir.AluOpType.mult,
                            mybir.AluOpType.add,
                        )
                    r0 = s0 + sub * S_SUB
                    nc.sync.dma_start(out=out[r0 : r0 + S_SUB, :], in_=osb[:])

    nc.compile()
    return nc


def _get_nc():
    key = "full"
    if key not in _cache:
        _cache[key] = build_nc()
    return _cache[key]


def kernel(x, weight_int8, scale, bias):
    global LAST_RESULT
    x = np.asarray(x, dtype=np.float32)
    w = np.asarray(weight_int8, dtype=np.int32)
    scale_f = np.float32(np.asarray(scale).reshape(()))
    bias = np.asarray(bias, dtype=np.float32)

    # host-side layout prep (sharding): contraction dim to the front
    xt = np.ascontiguousarray(x.reshape(S_TOT, IN_F).T)  # [in, s]
    wt_full = np.ascontiguousarray(w.T)  # [in, out]
    scale_rep = np.full((128, 1), scale_f, dtype=np.float32)

    nc = _get_nc()
    in_maps = []
    for c in range(NCORES):
        o0, o1 = c * OUT_PER, (c + 1) * OUT_PER
        in_maps.append(
            {
                "xt": xt,
                "wt": np.ascontiguousarray(wt_full[:, o0:o1]),
                "bias": np.ascontiguousarray(
                    np.broadcast_to(bias[o0:o1], (128, OUT_PER))
                ),
                "scale": scale_rep,
            }
        )

    res = run_bass_kernel_spmd(
        nc, in_maps, core_ids=list(range(NCORES)), trace=TRACE
    )
    LAST_RESULT = res
    out = np.concatenate([res.results[c]["out"] for c in range(NCORES)], axis=1)
    return out.reshape(B, S, OUT_F)


# revision 3
# speedup vs baseline: 1.0417x; 1.0417x over previous
"""CompressedLinear Trainium2 kernel.

Computes out[b,s,o] = x[b,s,i] @ (int8_weight[o,i] * scale).T + bias[o]
with x: [4,2048,4096] f32, weight_int8: [11008,4096] int32 (int8 values),
scale: scalar f32, bias: [11008] f32.

Sharding: column-parallel over 8 NeuronCores — each core owns 1376
out-features (weight + bias slice), x is replicated, outputs concat on
the last dim.

Per-core device kernel (Bass/Tile):
  - weight slice is uploaded in [in, out] layout (int32); the device
    dequantizes shard-locally: SWDGE cast-DMA int32 -> bf16 (exact for
    int8-range values) into a resident SBUF tile [4096 x 1376].
  - x is uploaded in [in, s] layout (f32); streamed as SWDGE cast-DMA
    f32 -> bf16 tiles.
  - TensorE: psum[s=128, o<=512] += xT_tile[k,s].T-free @ wT_tile[k,o]
    accumulated over 32 k-tiles of 128.
  - epilogue (DVE): out = psum * scale + bias in one scalar_tensor_tensor,
    then HWDGE store to DRAM in natural [s, o] layout.
"""

import numpy as np

import concourse.bacc as bacc
import concourse.mybir as mybir
import concourse.tile as tile
from concourse.bass_utils import run_bass_kernel_spmd

# Problem shape (hardcoded per contract)
B, S, IN_F, OUT_F = 4, 2048, 4096, 11008
NCORES = 8
OUT_PER = OUT_F // NCORES  # 1376
S_TOT = B * S  # 8192

# Tiling
KTILE = 128  # contraction per matmul
S_CHUNK = 512  # s-columns per x-load group
S_SUB = 128  # out-rows per psum block
KGRP = 4  # k-tiles per x DMA (1 MiB f32 reads)
NMAX = 512  # max moving free dim / psum bank

# set by test harness to capture profiles; harness calls kernel() untouched
TRACE = False
LAST_RESULT = None

_cache = {}


def _n_chunks(out_per):
    chunks = []
    off = 0
    while off < out_per:
        sz = min(NMAX, out_per - off)
        chunks.append((off, sz))
        off += sz
    return chunks


def build_nc(s_tot=S_TOT, in_f=IN_F, out_per=OUT_PER, s_chunk=S_CHUNK, kgrp=KGRP):
    f32 = mybir.dt.float32
    bf16 = mybir.dt.bfloat16
    i8 = mybir.dt.int8

    KT = in_f // KTILE  # k-tiles
    NKG = KT // kgrp  # x-load groups per s-chunk
    chunks = _n_chunks(out_per)

    nc = bacc.Bacc("TRN2", target_bir_lowering=False, debug=False, num_devices=NCORES)

    xt = nc.dram_tensor("xt", [in_f, s_tot], f32, kind="ExternalInput").ap()
    wt = nc.dram_tensor("wt", [in_f, out_per], i8, kind="ExternalInput").ap()
    bias = nc.dram_tensor("bias", [128, out_per], f32, kind="ExternalInput").ap()
    scale = nc.dram_tensor("scale", [128, 1], f32, kind="ExternalInput").ap()
    out = nc.dram_tensor("out", [s_tot, out_per], f32, kind="ExternalOutput").ap()

    with tile.TileContext(nc) as tc:
        with (
            tc.tile_pool(name="wt", bufs=1) as wt_pool,
            tc.tile_pool(name="xbf", bufs=2 * NKG) as xbf_pool,
            tc.tile_pool(name="psum", bufs=2, space="PSUM") as psum_pool,
            tc.tile_pool(name="osb", bufs=3) as osb_pool,
            tc.tile_pool(name="consts", bufs=1) as const_pool,
        ):
            bias_sb = const_pool.tile([128, out_per], f32, tag="bias", name="bias_sb")
            nc.sync.dma_start(out=bias_sb[:], in_=bias[:, :])
            scale_sb = const_pool.tile([128, 1], f32, tag="scale", name="scale_sb")
            nc.sync.dma_start(out=scale_sb[:], in_=scale[:, :])

            # Startup: interleave weight dequant (int8 -> bf16 cast DMA, exact
            # for int8-range values) with the first s-chunk's x loads so the
            # tensor engine starts within a few us.
            wts = [
                wt_pool.tile([128, out_per], bf16, tag=f"wt{k}", name=f"wt{k}")
                for k in range(KT)
            ]
            xg0 = []
            for g in range(NKG):
                for k in range(g * kgrp, (g + 1) * kgrp):
                    nc.gpsimd.dma_start(
                        out=wts[k][:], in_=wt[k * 128 : (k + 1) * 128, :]
                    )
                t = xbf_pool.tile([128, kgrp, s_chunk], bf16, tag="xbf", name=f"x0_{g}")
                src = xt[
                    g * kgrp * 128 : (g + 1) * kgrp * 128, 0:s_chunk
                ].rearrange("(g p) s -> p g s", p=128)
                nc.gpsimd.dma_start(out=t[:], in_=src)
                xg0.append(t)

            for ci, s0 in enumerate(range(0, s_tot, s_chunk)):
                if ci == 0:
                    xg = xg0
                else:
                    # x chunk load: cast f32 -> bf16 in DMA, [128, kgrp, s_chunk]
                    xg = []
                    for g in range(NKG):
                        t = xbf_pool.tile(
                            [128, kgrp, s_chunk], bf16, tag="xbf", name=f"x{ci}_{g}"
                        )
                        src = xt[
                            g * kgrp * 128 : (g + 1) * kgrp * 128, s0 : s0 + s_chunk
                        ].rearrange("(g p) s -> p g s", p=128)
                        nc.gpsimd.dma_start(out=t[:], in_=src)
                        xg.append(t)

                for sub in range(s_chunk // S_SUB):
                    psums = [
                        psum_pool.tile(
                            [128, NMAX], f32, tag=f"ps{j}", name=f"ps{ci}_{sub}_{j}"
                        )
                        for j in range(len(chunks))
                    ]
                    for k in range(KT):
                        lhsT = xg[k // kgrp][:, k % kgrp, sub * 128 : (sub + 1) * 128]
                        for j, (off, sz) in enumerate(chunks):
                            nc.tensor.matmul(
                                psums[j][:, :sz],
                                lhsT,
                                wts[k][:, off : off + sz],
                                start=(k == 0),
                                stop=(k == KT - 1),
                            )
                    osb = osb_pool.tile(
                        [128, out_per], f32, tag="osb", name=f"o{ci}_{sub}"
                    )
                    for j, (off, sz) in enumerate(chunks):
                        nc.vector.scalar_tensor_tensor(
                            osb[:, off : off + sz],
                            psums[j][:, :sz],
                            scale_sb[:, 0:1],
                            bias_sb[:, off : off + sz],
                            mybir.AluOpType.mult,
                            mybir.AluOpType.add,
                        )
                    r0 = s0 + sub * S_SUB
                    nc.sync.dma_start(out=out[r0 : r0 + S_SUB, :], in_=osb[:])

    nc.compile()
    return nc


def _get_nc():
    key = "full"
    if key not in _cache:
        _cache[key] = build_nc()
    return _cache[key]


def kernel(x, weight_int8, scale, bias):
    global LAST_RESULT
    x = np.asarray(x, dtype=np.float32)
    w = np.asarray(weight_int8)
    scale_f = np.float32(np.asarray(scale).reshape(()))
    bias = np.asarray(bias, dtype=np.float32)

    # host-side layout prep (sharding): contraction dim to the front; the
    # int8-valued weight is shipped in its compressed (int8) form
    xt = np.ascontiguousarray(x.reshape(S_TOT, IN_F).T)  # [in, s]
    wt_full = np.ascontiguousarray(w.T.astype(np.int8))  # [in, out]
    scale_rep = np.full((128, 1), scale_f, dtype=np.float32)

    nc = _get_nc()
    in_maps = []
    for c in range(NCORES):
        o0, o1 = c * OUT_PER, (c + 1) * OUT_PER
        in_maps.append(
            {
                "xt": xt,
                "wt": np.ascontiguousarray(wt_full[:, o0:o1]),
                "bias": np.ascontiguousarray(
                    np.broadcast_to(bias[o0:o1], (128, OUT_PER))
                ),
                "scale": scale_rep,
            }
        )

    res = run_bass_kernel_spmd(
        nc, in_maps, core_ids=list(range(NCORES)), trace=TRACE
    )
    LAST_RESULT = res
    out = np.concatenate([res.results[c]["out"] for c in range(NCORES)], axis=1)
    return out.reshape(B, S, OUT_F)


# revision 4
# speedup vs baseline: 1.0511x; 1.0090x over previous
"""CompressedLinear Trainium2 kernel.

Computes out[b,s,o] = x[b,s,i] @ (int8_weight[o,i] * scale).T + bias[o]
with x: [4,2048,4096] f32, weight_int8: [11008,4096] int32 (int8 values),
scale: scalar f32, bias: [11008] f32.

Sharding: column-parallel over 8 NeuronCores — each core owns 1376
out-features (weight + bias slice), x is replicated, outputs concat on
the last dim.

Per-core device kernel (Bass/Tile):
  - weight slice is uploaded in [in, out] layout (int32); the device
    dequantizes shard-locally: SWDGE cast-DMA int32 -> bf16 (exact for
    int8-range values) into a resident SBUF tile [4096 x 1376].
  - x is uploaded in [in, s] layout (f32); streamed as SWDGE cast-DMA
    f32 -> bf16 tiles.
  - TensorE: psum[s=128, o<=512] += xT_tile[k,s].T-free @ wT_tile[k,o]
    accumulated over 32 k-tiles of 128.
  - epilogue (DVE): out = psum * scale + bias in one scalar_tensor_tensor,
    then HWDGE store to DRAM in natural [s, o] layout.
"""

import numpy as np

import concourse.bacc as bacc
import concourse.mybir as mybir
import concourse.tile as tile
from concourse.bass_utils import run_bass_kernel_spmd

# Problem shape (hardcoded per contract)
B, S, IN_F, OUT_F = 4, 2048, 4096, 11008
NCORES = 8
OUT_PER = OUT_F // NCORES  # 1376
S_TOT = B * S  # 8192

# Tiling
KTILE = 128  # contraction per matmul
S_CHUNK = 512  # s-columns per x-load group
S_SUB = 128  # out-rows per psum block
KGRP = 4  # k-tiles per x DMA (1 MiB f32 reads)
NMAX = 512  # max moving free dim / psum bank

# set by test harness to capture profiles; harness calls kernel() untouched
TRACE = False
LAST_RESULT = None

_cache = {}


def _n_chunks(out_per):
    chunks = []
    off = 0
    while off < out_per:
        sz = min(NMAX, out_per - off)
        chunks.append((off, sz))
        off += sz
    return chunks


def build_nc(s_tot=S_TOT, in_f=IN_F, out_per=OUT_PER, s_chunk=S_CHUNK, kgrp=KGRP):
    f32 = mybir.dt.float32
    bf16 = mybir.dt.bfloat16
    i8 = mybir.dt.int8

    KT = in_f // KTILE  # k-tiles
    NKG = KT // kgrp  # x-load groups per s-chunk
    chunks = _n_chunks(out_per)

    nc = bacc.Bacc("TRN2", target_bir_lowering=False, debug=False, num_devices=NCORES)

    xt = nc.dram_tensor("xt", [in_f, s_tot], f32, kind="ExternalInput").ap()
    wt = nc.dram_tensor("wt", [in_f, out_per], i8, kind="ExternalInput").ap()
    bias = nc.dram_tensor("bias", [128, out_per], f32, kind="ExternalInput").ap()
    scale = nc.dram_tensor("scale", [128, 1], f32, kind="ExternalInput").ap()
    out = nc.dram_tensor("out", [s_tot, out_per], f32, kind="ExternalOutput").ap()

    # s-chunk schedule: two narrow warmup chunks so the first psum block
    # isn't gated on the full 8 MB x-chunk + 5.6 MB weight load.
    warm = min(s_chunk // 2, 256)
    if s_tot > 2 * warm and (s_tot - 2 * warm) % s_chunk == 0:
        chunk_sched = [warm, warm] + [s_chunk] * ((s_tot - 2 * warm) // s_chunk)
    else:
        chunk_sched = [s_chunk] * (s_tot // s_chunk)

    with tile.TileContext(nc) as tc:
        with (
            tc.tile_pool(name="wt", bufs=1) as wt_pool,
            tc.tile_pool(name="xbf", bufs=2 * NKG) as xbf_pool,
            tc.tile_pool(name="psum", bufs=2, space="PSUM") as psum_pool,
            tc.tile_pool(name="osb", bufs=3) as osb_pool,
            tc.tile_pool(name="consts", bufs=1) as const_pool,
        ):
            # Startup: interleave weight dequant (int8 -> bf16 cast DMA, exact
            # for int8-range values) with the first s-chunk's x loads, x tile
            # first — the tensor engine needs (xg0, wtg0) for its first MM.
            wts = [
                wt_pool.tile([128, kgrp, out_per], bf16, tag=f"wt{g}", name=f"wt{g}")
                for g in range(NKG)
            ]
            sc0 = chunk_sched[0]
            xg0 = []
            for g in range(NKG):
                t = xbf_pool.tile([128, kgrp, sc0], bf16, tag="xbf", name=f"x0_{g}")
                src = xt[g * kgrp * 128 : (g + 1) * kgrp * 128, 0:sc0].rearrange(
                    "(g p) s -> p g s", p=128
                )
                nc.gpsimd.dma_start(out=t[:], in_=src)
                xg0.append(t)
                wsrc = wt[g * kgrp * 128 : (g + 1) * kgrp * 128, :].rearrange(
                    "(g p) o -> p g o", p=128
                )
                nc.gpsimd.dma_start(out=wts[g][:], in_=wsrc)

            scale_sb = const_pool.tile([128, 1], f32, tag="scale", name="scale_sb")
            nc.sync.dma_start(out=scale_sb[:], in_=scale[:, :])
            bias_sb = const_pool.tile([128, out_per], f32, tag="bias", name="bias_sb")
            nc.sync.dma_start(out=bias_sb[:], in_=bias[:, :])

            s0 = 0
            for ci, sc in enumerate(chunk_sched):
                if ci == 0:
                    xg = xg0
                else:
                    # x chunk load: cast f32 -> bf16 in DMA, [128, kgrp, sc]
                    xg = []
                    for g in range(NKG):
                        t = xbf_pool.tile(
                            [128, kgrp, sc], bf16, tag="xbf", name=f"x{ci}_{g}"
                        )
                        src = xt[
                            g * kgrp * 128 : (g + 1) * kgrp * 128, s0 : s0 + sc
                        ].rearrange("(g p) s -> p g s", p=128)
                        nc.gpsimd.dma_start(out=t[:], in_=src)
                        xg.append(t)

                for sub in range(sc // S_SUB):
                    psums = [
                        psum_pool.tile(
                            [128, NMAX], f32, tag=f"ps{j}", name=f"ps{ci}_{sub}_{j}"
                        )
                        for j in range(len(chunks))
                    ]
                    for k in range(KT):
                        lhsT = xg[k // kgrp][:, k % kgrp, sub * 128 : (sub + 1) * 128]
                        for j, (off, sz) in enumerate(chunks):
                            nc.tensor.matmul(
                                psums[j][:, :sz],
                                lhsT,
                                wts[k // kgrp][:, k % kgrp, off : off + sz],
                                start=(k == 0),
                                stop=(k == KT - 1),
                            )
                    osb = osb_pool.tile(
                        [128, out_per], f32, tag="osb", name=f"o{ci}_{sub}"
                    )
                    for j, (off, sz) in enumerate(chunks):
                        nc.vector.scalar_tensor_tensor(
                            osb[:, off : off + sz],
                            psums[j][:, :sz],
                            scale_sb[:, 0:1],
                            bias_sb[:, off : off + sz],
                            mybir.AluOpType.mult,
                            mybir.AluOpType.add,
                        )
                    r0 = s0 + sub * S_SUB
                    nc.sync.dma_start(out=out[r0 : r0 + S_SUB, :], in_=osb[:])
                s0 += sc

    nc.compile()
    return nc


def _get_nc():
    key = "full"
    if key not in _cache:
        _cache[key] = build_nc()
    return _cache[key]


def kernel(x, weight_int8, scale, bias):
    global LAST_RESULT
    x = np.asarray(x, dtype=np.float32)
    w = np.asarray(weight_int8)
    scale_f = np.float32(np.asarray(scale).reshape(()))
    bias = np.asarray(bias, dtype=np.float32)

    # host-side layout prep (sharding): contraction dim to the front; the
    # int8-valued weight is shipped in its compressed (int8) form
    xt = np.ascontiguousarray(x.reshape(S_TOT, IN_F).T)  # [in, s]
    wt_full = np.ascontiguousarray(w.T.astype(np.int8))  # [in, out]
    scale_rep = np.full((128, 1), scale_f, dtype=np.float32)

    nc = _get_nc()
    in_maps = []
    for c in range(NCORES):
        o0, o1 = c * OUT_PER, (c + 1) * OUT_PER
        in_maps.append(
            {
                "xt": xt,
                "wt": np.ascontiguousarray(wt_full[:, o0:o1]),
                "bias": np.ascontiguousarray(
                    np.broadcast_to(bias[o0:o1], (128, OUT_PER))
                ),
                "scale": scale_rep,
            }
        )

    res = run_bass_kernel_spmd(
        nc, in_maps, core_ids=list(range(NCORES)), trace=TRACE
    )
    LAST_RESULT = res
    out = np.concatenate([res.results[c]["out"] for c in range(NCORES)], axis=1)
    return out.reshape(B, S, OUT_F)
